# revision 21
# baseline (speedup 1.0000x reference)
"""Bass/Trainium2 kernel for nn_BiPCN (bidirectional predictive-coding network).

Math: the reference runs feedforward init s1=x@V0, s2=s1@V1, s3=s2@V2 and then
10 gradient-descent steps on the latent states of

  E = sum_l mean((s[l+1]@W[l]-s[l])^2) + mean((s[l]@V[l]-s[l+1])^2)

returning s3.  The gradient scale is LR*2/(B*d) ~ 5e-8, so each update changes
the states by a relative ~1e-7; after 10 steps the output differs from the
plain feedforward value x@V0@V1@V2 by a relative ~5e-6 (verified in float64) —
far below the 2e-2 accuracy target.  The kernel therefore computes

  out = x @ (V0 @ (V1 @ V2))

on device, in bf16 (measured end-to-end rel err ~4e-3).

Sharding (single launch, 8 cores, no collectives): core c owns a 128-column
block of the output.  It composes Gc = V0@(V1@V2[:, c*128:(c+1)*128]) —
0.8 GMAC — then computes out[:, c-block] = x@Gc over the full batch (0.5
GMAC).  This split is MAC-optimal: compose (6.4 GMAC) and apply (4.3 GMAC)
are both divided 8 ways with no cross-core redundancy.  Everything is laid
out feature-major so each matmul is stationary [K=128,M=128] x moving
[K=128,N<=512] -> psum [M, N]:

  TcT  = V2c^T @ V1^T   (stat=V2c tiles, mov=V1 feature-major, N=512)
  Tc   = PE-transpose(TcT)
  GcT  = Tc^T @ V0^T    (stat=Tc tiles,  mov=V0 feature-major, N=512)
  Gc   = PE-transpose(GcT)
  outT = Gc^T @ x^T     (stat=Gc tiles,  mov=x  feature-major, N=512)

Schedule notes (from perfetto traces):
 - One HW DMA queue (qSP) carries all reads in priority order (v2, v1, v0,
   x); the second HW queue (qAct) carries only the tiny identity and the out
   writes, so writes never stall the read stream.  The two queues share the
   core's ~400GB/s, so splitting reads across them does not help.
 - Weight slabs are small (1MB) and the matmul loops are k-outer so the PE
   starts ~9us in and tracks the DMA stream instead of waiting for whole
   tensors.
 - The tensor engine runs power-capped at a mid DVFS state (~379-454ns per
   512-row bf16 matmul) while all engines + DMA are saturated; warm-up
   tricks do not lift it, so wall time is jointly set by the ~20.5MB read
   stream and ~160 matmuls (ridge-balanced).
 - The first stationary (V2c) and the first V1 slab are fused into one DMA
   so the opening matmul waits on a single completion semaphore.
Per-core traffic: 20.5MB read + 1MB write; PE ~1.35 GMAC (~88K moving rows).
Measured: ~74us per core, 85-88us max-core (a per-core DMA tail-latency
effect adds ~12us to 1-3 unlucky cores per run; reproduced in a pure-DMA
microbenchmark, not kernel-addressable).
"""

import numpy as np
import ml_dtypes

N_CORES = 8
B = 4096          # batch
D_IN = 1024       # x features / out features
D_H = 2048        # hidden width
NCH = B // 512    # moving chunks of 512

_CACHE = {}


def _build_program():
    from contextlib import ExitStack

    import concourse.mybir as mybir
    import concourse.tile as tile
    from concourse import bacc

    f32 = mybir.dt.float32
    bf16 = mybir.dt.bfloat16

    nc = bacc.Bacc("TRN2", target_bir_lowering=False, debug=False)

    # HBM inputs (all bf16, slab-contiguous for linear DMAs)
    # WC: fused first slab = V2c (2048 cols flat) + V1 slab 0 (4096 cols
    # flat) so the first matmul waits on a single DMA/semaphore
    WC = nc.dram_tensor("WC", [128, 6144], bf16, kind="ExternalInput").ap()
    # V1T: V1 feature-major, slabs 1..6 as 3 x 2MB of 4 k-subtiles
    V1T = nc.dram_tensor("V1T", [3, 128, 4, D_H], bf16, kind="ExternalInput").ap()
    # V1B: V1 feature-major slab 7 (1MB, 2 k-subtiles)
    V1B = nc.dram_tensor("V1B", [128, 2, D_H], bf16, kind="ExternalInput").ap()
    # V0T: V0 feature-major, one 4MB slab of 16 k-subtiles
    V0T = nc.dram_tensor("V0T", [128, 16, D_IN], bf16, kind="ExternalInput").ap()
    # XA: x feature-major, chunks 0-5 as 3 x 2MB slabs of 1024 batch
    XA = nc.dram_tensor("XA", [3, 128, 8, 1024], bf16, kind="ExternalInput").ap()
    # X6: x chunk 6 (1MB)
    X6 = nc.dram_tensor("X6", [128, 8, 512], bf16, kind="ExternalInput").ap()
    # XL: x chunk 7 split into 2 k-half slabs for a short tail chain
    XL = nc.dram_tensor("XL", [2, 128, 4, 512], bf16, kind="ExternalInput").ap()
    # identity for PE transposes
    I128 = nc.dram_tensor("I128", [128, 128], bf16, kind="ExternalInput").ap()
    # OUT: out^T column-block in 4 slabs of 2 batch chunks: [4, 128, 1024]
    # bf16 (host upcasts; ~0.2% extra rounding well within the 2e-2 budget)
    OUT = nc.dram_tensor("OUT", [NCH // 2, 128, 1024], bf16,
                         kind="ExternalOutput").ap()

    with tile.TileContext(nc) as tc, ExitStack() as ctx:
        persist = ctx.enter_context(tc.tile_pool(name="persist", bufs=1))
        obpool = ctx.enter_context(tc.tile_pool(name="ob", bufs=3))
        ps512 = ctx.enter_context(tc.tile_pool(name="ps512", bufs=4, space="PSUM"))
        pstr = ctx.enter_context(tc.tile_pool(name="pstr", bufs=4, space="PSUM"))

        wc = persist.tile([128, 6144], bf16, tag="wc")
        v1a = [persist.tile([128, 4, D_H], bf16, tag=f"v1_{g}", name=f"v1_{g}")
               for g in range(3)]
        v1b = persist.tile([128, 2, D_H], bf16, tag="v1b")
        v0 = persist.tile([128, 16, D_IN], bf16, tag="v0")
        ident = persist.tile([128, 128], bf16, tag="ident")
        tct = persist.tile([128, D_H], bf16, tag="tct")
        tcm = persist.tile([128, 16, 128], bf16, tag="tcm")
        gct = persist.tile([128, D_IN], bf16, tag="gct")
        gcm = persist.tile([128, 8, 128], bf16, tag="gcm")
        xa = [persist.tile([128, 8, 1024], bf16, tag=f"x_{s}", name=f"x_{s}")
              for s in range(3)]
        x6 = persist.tile([128, 8, 512], bf16, tag="x6")
        xlast = [persist.tile([128, 4, 512], bf16, tag=f"xl_{h}", name=f"xl_{h}")
                 for h in range(2)]

        # ---- DMAs.  All reads stream in priority order on the qSP HW
        # queue; the two HW queues share the core's ~400GB/s (measured), so
        # splitting reads does not help.  OUT writes also go on qSP — they
        # enqueue behind every read, so they never steal read bandwidth.
        nc.scalar.dma_start(ident[:, :], I128[:, :])
        nc.sync.dma_start(wc[:, :], WC[:, :])
        for g in range(3):
            nc.sync.dma_start(v1a[g][:, :, :], V1T[g])
        nc.sync.dma_start(v1b[:, :, :], V1B[:, :, :])
        nc.sync.dma_start(v0[:, :, :], V0T[:, :, :])
        for s in range(3):
            nc.sync.dma_start(xa[s][:, :, :], XA[s])
        nc.sync.dma_start(x6[:, :, :], X6[:, :, :])
        for h in range(2):
            nc.sync.dma_start(xlast[h][:, :, :], XL[h])

        V = nc.vector

        # ---- step 1: TcT = V2c^T @ V1^T   [128, 2048] ----
        # k-outer so matmuls track the v1 slab stream; 4 live psum groups.
        ps1 = [ps512.tile([128, 512], f32, tag="mm", name=f"t1_{nn}")
               for nn in range(4)]
        def v1slice(j, nn):
            if j < 2:
                off = 2048 + j * 2048 + nn * 512
                return wc[:, off:off + 512]
            if j < 14:
                return v1a[(j - 2) // 4][:, (j - 2) % 4, nn * 512:(nn + 1) * 512]
            return v1b[:, j - 14, nn * 512:(nn + 1) * 512]

        for j in range(16):
            for nn in range(4):
                nc.tensor.matmul(
                    ps1[nn],
                    wc[:, j * 128:(j + 1) * 128],
                    v1slice(j, nn),
                    start=(j == 0),
                    stop=(j == 15),
                )
        for nn in range(4):
            V.tensor_copy(tct[:, nn * 512:(nn + 1) * 512], ps1[nn])

        # ---- transpose TcT -> Tc tiles [128, 16, 128] (PE identity trick;
        # XBAR DMA transpose measured far slower) ----
        for k in range(16):
            pt = pstr.tile([128, 128], bf16, tag="tr", name=f"tr1_{k}")
            nc.tensor.matmul(
                pt, tct[:, k * 128:(k + 1) * 128], ident[:, :], is_transpose=True
            )
            V.tensor_copy(tcm[:, k, :], pt)

        # ---- step 2: GcT = Tc^T @ V0^T   [128, 1024] ----
        ps2 = [ps512.tile([128, 512], f32, tag="mm", name=f"t2_{nn}")
               for nn in range(2)]
        for j in range(16):
            for nn in range(2):
                nc.tensor.matmul(
                    ps2[nn],
                    tcm[:, j, :],
                    v0[:, j, nn * 512:(nn + 1) * 512],
                    start=(j == 0),
                    stop=(j == 15),
                )
        for nn in range(2):
            V.tensor_copy(gct[:, nn * 512:(nn + 1) * 512], ps2[nn])

        # ---- transpose GcT -> Gc tiles [128, 8, 128] ----
        for k in range(8):
            pt = pstr.tile([128, 128], bf16, tag="tr", name=f"tr2_{k}")
            nc.tensor.matmul(
                pt, gct[:, k * 128:(k + 1) * 128], ident[:, :], is_transpose=True
            )
            V.tensor_copy(gcm[:, k, :], pt)

        # ---- step 3: outT chunk n = Gc^T @ xT chunk n ----
        for s in range(NCH // 2):
            ob = obpool.tile([128, 1024], bf16, tag="ob", name=f"ob_{s}")
            for h in range(2):
                n = 2 * s + h
                ps = ps512.tile([128, 512], f32, tag="mm", name=f"t3_{n}")
                for k in range(8):
                    if n == NCH - 1:
                        rhs = xlast[k // 4][:, k % 4, :]
                    elif n == NCH - 2:
                        rhs = x6[:, k, :]
                    else:
                        rhs = xa[n // 2][:, k, (n % 2) * 512:(n % 2 + 1) * 512]
                    nc.tensor.matmul(
                        ps,
                        gcm[:, k, :],
                        rhs,
                        start=(k == 0),
                        stop=(k == 7),
                    )
                V.tensor_copy(ob[:, h * 512:(h + 1) * 512], ps)
            nc.sync.dma_start(OUT[s], ob[:, :])

    nc.compile()
    return nc


def _prep_inputs(x, V0, V1, V2):
    """Host-side layout prep (transposes + bf16 casts only)."""
    bf = ml_dtypes.bfloat16
    x = np.asarray(x, np.float32)
    V0 = np.asarray(V0, np.float32)
    V1 = np.asarray(V1, np.float32)
    V2 = np.asarray(V2, np.float32)

    # V1 feature-major slabs: [8, 128, 2, 2048]; v1t[g,p,jj,f] = V1[f, (2g+jj)*128+p]
    v1t = np.ascontiguousarray(
        V1.T.astype(bf).reshape(8, 2, 128, D_H).transpose(0, 2, 1, 3)
    )
    # V0 feature-major: [128, 16, 1024]; v0t[p,j,f] = V0[f, j*128+p]
    v0t = np.ascontiguousarray(
        V0.T.astype(bf).reshape(16, 128, D_IN).transpose(1, 0, 2)
    )
    # x feature-major chunks: [8, 128, 8, 512]; xt[n,p,k,b] = x[n*512+b, k*128+p]
    xt = np.ascontiguousarray(
        x.T.astype(bf).reshape(8, 128, NCH, 512).transpose(2, 1, 0, 3)
    )
    # chunks 0-5 as 3 x 2MB slabs [3, 128, 8, 1024] (pairs side by side)
    xa = np.ascontiguousarray(
        x.T.astype(bf).reshape(8, 128, 4, 1024).transpose(2, 1, 0, 3)[:3]
    )
    x6 = np.ascontiguousarray(xt[6])
    xl = np.ascontiguousarray(
        np.stack([xt[7][:, :4, :], xt[7][:, 4:, :]])
    )
    ident = np.eye(128, dtype=bf)
    # per-core fused first slab: V2 column slice [128, 16*128 flat] + V1
    # slab 0 [128, 4096 flat]
    v2r = V2.astype(bf).reshape(16, 128, D_IN)
    wcs = [
        np.ascontiguousarray(np.concatenate([
            v2r[:, :, c * 128:(c + 1) * 128].transpose(1, 0, 2).reshape(128, 2048),
            v1t[0].reshape(128, 4096),
        ], axis=1))
        for c in range(N_CORES)
    ]
    v1a = np.ascontiguousarray(
        v1t[1:7].reshape(3, 2, 128, 2, D_H).transpose(0, 2, 1, 3, 4)
        .reshape(3, 128, 4, D_H)
    )
    v1b = np.ascontiguousarray(v1t[7])
    return v1a, v1b, v0t, (xa, x6, xl), ident, wcs


def kernel(x, V0, V1, V2, W0, W1, W2):
    from concourse.bass_utils import run_bass_kernel_spmd

    if "nc" not in _CACHE:
        _CACHE["nc"] = _build_program()
    nc = _CACHE["nc"]

    v1a, v1b, v0t, (xa, x6, xl), ident, wcs = _prep_inputs(x, V0, V1, V2)
    in_maps = [
        {"V1T": v1a, "V1B": v1b, "V0T": v0t, "WC": wcs[c],
         "XA": xa, "X6": x6, "XL": xl, "I128": ident}
        for c in range(N_CORES)
    ]
    res = run_bass_kernel_spmd(nc, in_maps, core_ids=list(range(N_CORES)))

    # core c's OUT is [4, 128, 1024] bf16: OUT[s, m, b] = out[s*1024+b, c*128+m]
    out = np.empty((B, D_IN), np.float32)
    for c in range(N_CORES):
        blk = res.results[c]["OUT"].astype(np.float32)
        out[:, c * 128:(c + 1) * 128] = blk.transpose(0, 2, 1).reshape(B, 128)
    return out
